# revision 10
# baseline (speedup 1.0000x reference)
"""Haar DWT2 level-1 kernel for Trainium2 (Bass/Tile), 8-core data parallel.

Input:  image_tensor (16, 4, 1024, 1024) f32
Output: (LL (16, 4, 512, 512), high (16, 12, 512, 512))  [high = concat(LH, HL, HH, axis=1)]

Sharding: batch dim across 8 cores (2 batches = 8 images of 1024x1024 per core).

Per-core algorithm, per half-image (256 rows of row-pairs handled per iteration):
  - DMA rows in pairs: partition p holds rows (256g + 2p) and (256g + 2p + 1)
    contiguously (8 KB chunks -> efficient descriptors).
  - ACT: in-place prescale X *= 0.5 (exact in f32).
  - DVE: S = even_row + odd_row, D = even_row - odd_row        (row combine)
  - DVE: LL = S[0::2]+S[1::2], HL = S[0::2]-S[1::2],
         LH = D[0::2]+D[1::2], HH = D[0::2]-D[1::2]            (col combine)
  - DMA out the 4 half-size planes.
"""

import os
import sys

import numpy as np

for _p in ("/opt/trn_rl_repo",):
    if os.path.isdir(_p) and _p not in sys.path:
        sys.path.insert(0, _p)

B, C, H, W = 16, 4, 1024, 1024
N_CORES = 8
BPC = B // N_CORES          # batches per core
IMGS = BPC * C              # images per core
P = 128                     # partitions
GROUPS = 2                  # row-pair groups per iteration (256 rows)
ROWS_PER_ITER = GROUPS * 2 * P   # 512
ITERS_PER_IMG = H // ROWS_PER_ITER

_nc_cache = None


def _build(imgs: int = IMGS, passes: int = 1, internal_io: bool = False):
    from concourse import bacc, mybir
    from concourse.tile import TileContext

    f32 = mybir.dt.float32
    # Bacc (not raw Bass): its compile() runs generate_event_semaphores, which
    # splits multi-wait sync conditions — TRN2 DMA instructions allow only one.
    nc = bacc.Bacc("TRN2", target_bir_lowering=False, name="dwt2_haar")

    if internal_io:
        # Timing-only variant: all big tensors Internal so the per-call axon
        # transport is tiny; values are garbage, throughput is identical.
        x = nc.dram_tensor("x", [imgs, H, W], f32, kind="Internal")
        ll = nc.dram_tensor("ll", [imgs, H // 2, W // 2], f32, kind="Internal")
        lh = nc.dram_tensor("lh", [imgs, H // 2, W // 2], f32, kind="Internal")
        hl = nc.dram_tensor("hl", [imgs, H // 2, W // 2], f32, kind="Internal")
        hh = nc.dram_tensor("hh", [imgs, H // 2, W // 2], f32, kind="Internal")
        tok_in = nc.dram_tensor("tok_in", [1, 4], f32, kind="ExternalInput")
        tok_out = nc.dram_tensor("tok_out", [1, 4], f32, kind="ExternalOutput")
    else:
        x = nc.dram_tensor("x", [imgs, H, W], f32, kind="ExternalInput")
        ll = nc.dram_tensor("ll", [imgs, H // 2, W // 2], f32, kind="ExternalOutput")
        lh = nc.dram_tensor("lh", [imgs, H // 2, W // 2], f32, kind="ExternalOutput")
        hl = nc.dram_tensor("hl", [imgs, H // 2, W // 2], f32, kind="ExternalOutput")
        hh = nc.dram_tensor("hh", [imgs, H // 2, W // 2], f32, kind="ExternalOutput")

    with TileContext(nc) as tc:
        with (
            tc.tile_pool(name="xp", bufs=3) as xpool,
            tc.tile_pool(name="sd", bufs=3) as sdpool,
            tc.tile_pool(name="op", bufs=3) as opool,
        ):
            for img in range(imgs * passes):
                img = img % imgs
                for it in range(ITERS_PER_IMG):
                    r0 = it * ROWS_PER_ITER
                    # X[p, g, 0:1024]   = x[img, r0 + 256g + 2p, :]
                    # X[p, g, 1024:2048] = x[img, r0 + 256g + 2p + 1, :]
                    X = xpool.tile([P, GROUPS, 2 * W], f32, tag="X", name="X")
                    src = x[img, r0 : r0 + ROWS_PER_ITER, :].rearrange(
                        "(g p two) w -> p g (two w)", g=GROUPS, p=P, two=2
                    )
                    nc.sync.dma_start(out=X[:], in_=src)

                    E = X[:, :, 0:W]
                    O = X[:, :, W : 2 * W]
                    S = sdpool.tile([P, GROUPS, W], f32, tag="S", name="S")
                    D = sdpool.tile([P, GROUPS, W], f32, tag="D", name="D")
                    nc.vector.tensor_add(out=S[:], in0=E, in1=O)
                    nc.vector.tensor_sub(out=D[:], in0=E, in1=O)
                    # halve in place on ACT (exact in f32); X stays DVE-only so
                    # its reload DMA needs a single sync wait.
                    nc.scalar.mul(S[:], S[:], 0.5)
                    nc.scalar.mul(D[:], D[:], 0.5)

                    LLt = opool.tile([P, GROUPS, W // 2], f32, tag="LL", name="LLt")
                    LHt = opool.tile([P, GROUPS, W // 2], f32, tag="LH", name="LHt")
                    HLt = opool.tile([P, GROUPS, W // 2], f32, tag="HL", name="HLt")
                    HHt = opool.tile([P, GROUPS, W // 2], f32, tag="HH", name="HHt")
                    nc.vector.tensor_add(out=LLt[:], in0=S[:, :, 0::2], in1=S[:, :, 1::2])
                    nc.vector.tensor_sub(out=HLt[:], in0=S[:, :, 0::2], in1=S[:, :, 1::2])
                    nc.vector.tensor_add(out=LHt[:], in0=D[:, :, 0::2], in1=D[:, :, 1::2])
                    nc.vector.tensor_sub(out=HHt[:], in0=D[:, :, 0::2], in1=D[:, :, 1::2])

                    # row-pair index = r0/2 + 128g + p
                    q0 = r0 // 2
                    for t, dst in ((LLt, ll), (LHt, lh), (HLt, hl), (HHt, hh)):
                        dsl = dst[img, q0 : q0 + GROUPS * P, :].rearrange(
                            "(g p) w -> p g w", g=GROUPS, p=P
                        )
                        nc.scalar.dma_start(out=dsl, in_=t[:])
            if internal_io:
                tok = xpool.tile([1, 4], f32, tag="tok", name="tok")
                nc.sync.dma_start(out=tok[:], in_=tok_in[:])
                nc.sync.dma_start(out=tok_out[:], in_=tok[:])
    # run Bacc's compile pipeline (register allocation, wait splitting, ...);
    # run_bass_via_pjrt serializes nc.m as-is and never finalizes.
    nc.finalize()
    return nc


def _get_nc():
    global _nc_cache
    if _nc_cache is None:
        _nc_cache = _build()
    return _nc_cache


def _run(image_tensor: np.ndarray, trace: bool = False):
    from concourse.bass_utils import run_bass_kernel_spmd

    image_tensor = np.ascontiguousarray(np.asarray(image_tensor, dtype=np.float32))
    assert image_tensor.shape == (B, C, H, W)

    nc = _get_nc()
    in_maps = [
        {"x": image_tensor[i * BPC : (i + 1) * BPC].reshape(IMGS, H, W)}
        for i in range(N_CORES)
    ]
    res = run_bass_kernel_spmd(nc, in_maps, core_ids=list(range(N_CORES)), trace=trace)

    LL = np.empty((B, C, H // 2, W // 2), np.float32)
    high = np.empty((B, 3 * C, H // 2, W // 2), np.float32)
    for i, r in enumerate(res.results):
        sl = slice(i * BPC, (i + 1) * BPC)
        LL[sl] = r["ll"].reshape(BPC, C, H // 2, W // 2)
        high[sl, 0:C] = r["lh"].reshape(BPC, C, H // 2, W // 2)
        high[sl, C : 2 * C] = r["hl"].reshape(BPC, C, H // 2, W // 2)
        high[sl, 2 * C : 3 * C] = r["hh"].reshape(BPC, C, H // 2, W // 2)
    return (LL, high), res


def kernel(image_tensor: np.ndarray):
    (LL, high), _ = _run(image_tensor)
    return LL, high
